# revision 12
# baseline (speedup 1.0000x reference)
"""Additive attention (Bahdanau) on 8 TRN2 NeuronCores, pure data-parallel.

Per-core shard: 8 batches. Host pre-work (sharding/layout only, no math):
cast encoder_outputs to bf16, drop masked positions (they provably do not
affect the reference output: their weights are exactly 0), transpose to
[E, n_kept] per batch so the device streams it with E on SBUF partitions.
Positions are padded to a static NIDX with zeros + mask=0.

Device per batch:
  projT[h,s] = W_h.T @ encT          (PE, W_h stationary, PSUM-accum over E)
  energyT    = tanh(projT + dec@W_d) (ScalarE, add fused as per-partition bias)
  scores     = v.T @ energyT         (PE)
  p          = exp(scores)           (ScalarE; no max-subtraction: scores O(1))
  p_m        = p * mask              (DVE fused multiply, bf16)
  ctx_num    = sum_s p_m[s]*encT[:,s]  (DVE STT-accum over the SAME
                SBUF-resident encT tiles, two s-halves -> 1 HBM pass)
Host post-work: normalize (softmax denominator), scatter packed weights back
to [B, S], reassemble context from the [128, EC] partials.
"""

import numpy as np
import ml_dtypes

B, S, E, H = 64, 2048, 1024, 512
N_CORES = 8
BPC = B // N_CORES        # batches per core
EC = E // 128             # e-chunks (8)
HC = H // 128             # h-chunks (4)
SW = 512                  # matmul moving free dim
NIDX = 1280               # padded kept-position count (mean 1024, sigma ~23)

BF16 = ml_dtypes.bfloat16

_cached = {}


def _build_nc(nidx):
    import concourse.bass as bass  # noqa: F401
    import concourse.tile as tile
    from concourse import bacc, mybir

    f32 = mybir.dt.float32
    bf16 = mybir.dt.bfloat16

    s_tiles = []
    off = 0
    while off < nidx:
        w = min(SW, nidx - off)
        s_tiles.append((off, w))
        off += w
    # two context halves, split on an s_tile boundary near the middle
    n_first = max(1, len(s_tiles) - 1)
    half_edges = [(0, s_tiles[n_first - 1][0] + s_tiles[n_first - 1][1])]
    half_edges.append((half_edges[0][1], nidx))

    nc = bacc.Bacc("TRN2", target_bir_lowering=False, debug=False,
                   num_devices=N_CORES)

    enc_d = nc.dram_tensor("encT", [BPC, E, nidx], bf16, kind="ExternalInput")
    wh_d = nc.dram_tensor("w_h", [E, H], bf16, kind="ExternalInput")
    wd_d = nc.dram_tensor("w_d", [H, H], f32, kind="ExternalInput")
    decT_d = nc.dram_tensor("decT", [H, BPC], f32, kind="ExternalInput")
    v_d = nc.dram_tensor("v_col", [128, HC], bf16, kind="ExternalInput")
    mask_d = nc.dram_tensor("mask_f", [BPC, nidx], f32, kind="ExternalInput")
    octx_d = nc.dram_tensor("out_ctx", [BPC, 2, 128, EC], f32,
                            kind="ExternalOutput")
    ow_d = nc.dram_tensor("out_w", [BPC, nidx], bf16, kind="ExternalOutput")

    with tile.TileContext(nc) as tc, \
         tc.tile_pool(name="const", bufs=1) as const_pool, \
         tc.tile_pool(name="enc", bufs=2) as enc_pool, \
         tc.tile_pool(name="energy", bufs=3) as energy_pool, \
         tc.tile_pool(name="small", bufs=2) as small_pool, \
         tc.tile_pool(name="wb", bufs=2) as wb_pool, \
         tc.tile_pool(name="psA", bufs=4, space="PSUM") as psA, \
         tc.tile_pool(name="psB", bufs=2, space="PSUM") as psB:

        # ---- batch-0 data first so PE can start ASAP ----
        enc_g0 = enc_pool.tile([128, EC, nidx], bf16, tag="enc", name="enc_0")
        for ec in range(EC):
            nc.sync.dma_start(enc_g0[:, ec, :], enc_d[0, ec * 128:(ec + 1) * 128, :])
        w_sb = const_pool.tile([128, EC, H], bf16, tag="w_sb")
        nc.sync.dma_start(w_sb[:], wh_d.ap().rearrange("(c p) h -> p c h", p=128))

        # ---- remaining constants ----
        wd_sb = const_pool.tile([128, HC, H], f32, tag="wd_sb")
        nc.sync.dma_start(wd_sb[:], wd_d.ap().rearrange("(c p) h -> p c h", p=128))
        decT_sb = const_pool.tile([128, HC, BPC], f32, tag="decT_sb")
        nc.sync.dma_start(decT_sb[:], decT_d.ap().rearrange("(c p) b -> p c b", p=128))
        v_sb = const_pool.tile([128, HC], bf16, tag="v_sb")
        nc.sync.dma_start(v_sb[:], v_d[:, :])

        # ---- proj_dec = (dec @ W_d)^T  -> [128, hc, b] (bias for tanh) ----
        pdT_sb = const_pool.tile([128, HC, BPC], f32, tag="pdT_sb")
        for hc in range(HC):
            pd_ps = psB.tile([128, BPC], f32, tag="pd", bufs=1)
            for dc in range(HC):
                nc.tensor.matmul(
                    pd_ps[:],
                    lhsT=wd_sb[:, dc, hc * 128:(hc + 1) * 128],
                    rhs=decT_sb[:, dc, :],
                    start=(dc == 0), stop=(dc == HC - 1),
                )
            nc.scalar.copy(pdT_sb[:, hc, :], pd_ps[:])

        # ---- per batch ----
        for b in range(BPC):
            if b == 0:
                enc_g = enc_g0
            else:
                enc_g = enc_pool.tile([128, EC, nidx], bf16, tag="enc",
                                      name=f"enc_{b}")
                for ec in range(EC):
                    nc.sync.dma_start(enc_g[:, ec, :],
                                      enc_d[b, ec * 128:(ec + 1) * 128, :])

            mask_t = small_pool.tile([1, nidx], f32, tag="mask")
            nc.sync.dma_start(mask_t[:], mask_d[b:b + 1, :])

            p_mb = small_pool.tile([1, nidx], bf16, tag="p_mb")
            ctx = small_pool.tile([128, 2, EC], f32, tag="ctx")

            # proj + tanh (per s-tile) -> energyT -> scores -> exp
            p_exp = small_pool.tile([1, nidx], f32, tag="p_exp")
            done_tiles = 0
            for half, (h0, h1) in enumerate(half_edges):
                hw = h1 - h0
                while done_tiles < len(s_tiles) and \
                        s_tiles[done_tiles][0] < h1:
                    s0, sw = s_tiles[done_tiles]
                    st = done_tiles
                    done_tiles += 1
                    energy = energy_pool.tile([128, HC, sw], bf16, tag="energy",
                                              name=f"energy_{b}_{st}",
                                              padded_shape=[128, HC, SW])
                    for hc in range(HC):
                        pj = psA.tile([128, sw], f32, tag="pj",
                                      padded_shape=[128, SW])
                        for ec in range(EC):
                            nc.tensor.matmul(
                                pj[:],
                                lhsT=w_sb[:, ec, hc * 128:(hc + 1) * 128],
                                rhs=enc_g[:, ec, s0:s0 + sw],
                                start=(ec == 0), stop=(ec == EC - 1),
                            )
                        nc.scalar.activation(
                            energy[:, hc, :], pj[:],
                            mybir.ActivationFunctionType.Tanh,
                            bias=pdT_sb[:, hc, b:b + 1], scale=1.0,
                        )
                    sc = psB.tile([1, sw], f32, tag="sc", padded_shape=[1, SW])
                    for hc in range(HC):
                        nc.tensor.matmul(
                            sc[:],
                            lhsT=v_sb[:, hc:hc + 1],
                            rhs=energy[:, hc, :],
                            start=(hc == 0), stop=(hc == HC - 1),
                        )
                    nc.scalar.activation(
                        p_exp[:, s0:s0 + sw], sc[:],
                        mybir.ActivationFunctionType.Exp,
                    )

                # masked unnormalized weights (bf16) for this half
                nc.vector.scalar_tensor_tensor(
                    out=p_mb[:, h0:h1], in0=p_exp[:, h0:h1], scalar=1.0,
                    in1=mask_t[:, h0:h1],
                    op0=mybir.AluOpType.bypass, op1=mybir.AluOpType.mult,
                )
                wb = wb_pool.tile([128, hw], bf16, tag=f"wb{half}",
                                  name=f"wb_{b}_{half}",
                                  padded_shape=[128, half_edges[0][1]])
                nc.gpsimd.partition_broadcast(wb[:], p_mb[:, h0:h1])

                scr = wb_pool.tile([128, hw], bf16, tag=f"scr{half}",
                                   name=f"scr_{b}_{half}",
                                   padded_shape=[128, half_edges[0][1]])
                for ec in range(EC):
                    nc.vector.scalar_tensor_tensor(
                        out=scr[:],
                        in0=enc_g[:, ec, h0:h1],
                        scalar=1.0,
                        in1=wb[:],
                        op0=mybir.AluOpType.bypass, op1=mybir.AluOpType.mult,
                        accum_out=ctx[:, half, ec:ec + 1],
                    )
                nc.sync.dma_start(octx_d[b, half], ctx[:, half, :])

            nc.sync.dma_start(ow_d[b:b + 1, :], p_mb[:])

    nc.finalize()
    return nc


def _get_nc(nidx):
    key = f"nc_{nidx}"
    if key not in _cached:
        _cached[key] = _build_nc(nidx)
    return _cached[key]


def _host_prep(decoder_hidden, encoder_outputs, mask, W_h, W_d, v, nidx):
    decoder_hidden = np.asarray(decoder_hidden, dtype=np.float32)
    encoder_outputs = np.asarray(encoder_outputs)
    W_h = np.asarray(W_h, dtype=np.float32)
    W_d = np.asarray(W_d, dtype=np.float32)
    v = np.asarray(v, dtype=np.float32)

    wh_bf = np.ascontiguousarray(W_h.astype(BF16))
    v_col = np.ascontiguousarray(v.reshape(HC, 128).T.astype(BF16))

    from concurrent.futures import ThreadPoolExecutor

    def make_core(c):
        sl = slice(c * BPC, (c + 1) * BPC)
        encc = encoder_outputs[sl]
        encT = np.zeros((BPC, E, nidx), dtype=BF16)
        mask_f = np.zeros((BPC, nidx), dtype=np.float32)
        kept_c = []
        for b in range(BPC):
            kept = np.flatnonzero(mask[c * BPC + b])
            n = len(kept)
            kept_c.append(kept)
            encT[b, :, :n] = encc[b, kept].astype(BF16).T
            mask_f[b, :n] = 1.0
        return {
            "encT": encT,
            "w_h": wh_bf,
            "w_d": W_d,
            "decT": np.ascontiguousarray(decoder_hidden[sl].T),
            "v_col": v_col,
            "mask_f": mask_f,
        }, kept_c

    with ThreadPoolExecutor(max_workers=8) as ex:
        out = list(ex.map(make_core, range(N_CORES)))
    in_maps = [o[0] for o in out]
    kept_all = [k for o in out for k in o[1]]
    return in_maps, kept_all


def kernel(decoder_hidden, encoder_outputs, mask, W_h, W_d, v, _trace=False):
    from concourse.bass_utils import run_bass_kernel_spmd

    mask = np.asarray(mask)
    max_kept = int((mask != 0).sum(axis=1).max())
    nidx = NIDX if max_kept <= NIDX else S
    nc = _get_nc(nidx)
    in_maps, kept_all = _host_prep(
        decoder_hidden, encoder_outputs, mask, W_h, W_d, v, nidx)
    res = run_bass_kernel_spmd(nc, in_maps, core_ids=list(range(N_CORES)),
                               trace=_trace)
    # ctx partials: [BPC, 2, 128, EC] -> sum halves -> [e = ec*128 + p]
    ctx_raw = np.concatenate(
        [res.results[i]["out_ctx"].reshape(BPC, 2, 128, EC)
         for i in range(N_CORES)], axis=0)
    ctx_num = ctx_raw.sum(axis=1)                      # [B, 128, EC]
    ctx_num = ctx_num.transpose(0, 2, 1).reshape(B, E)  # e = ec*128 + p
    w_packed = np.concatenate(
        [res.results[i]["out_w"].reshape(BPC, nidx).astype(np.float32)
         for i in range(N_CORES)], axis=0)
    lsum = w_packed.sum(axis=1, keepdims=True)
    context = (ctx_num / lsum).astype(np.float32)
    w_norm = w_packed / lsum
    attn = np.zeros((B, S), dtype=np.float32)
    for b in range(B):
        kept = kept_all[b]
        attn[b, kept] = w_norm[b, :len(kept)]
    if _trace:
        _cached["last_result"] = res
    return context, attn


# revision 17
# speedup vs baseline: 1.2905x; 1.2905x over previous
"""Additive attention (Bahdanau) on 8 TRN2 NeuronCores, pure data-parallel.

Per-core shard: 8 batches. Host pre-work (sharding/layout only, no math):
cast encoder_outputs to bf16, drop masked positions (they provably do not
affect the reference output: their weights are exactly 0), transpose to
[E, n_kept] per batch so the device streams it with E on SBUF partitions.
Positions are padded to a static NIDX with zeros + mask=0.

Device per batch:
  projT[h,s] = W_h.T @ encT          (PE, W_h stationary, PSUM-accum over E)
  energyT    = tanh(projT + dec@W_d) (ScalarE, add fused as per-partition bias)
  scores     = v.T @ energyT         (PE)
  p          = exp(scores)           (ScalarE; no max-subtraction: scores O(1))
  p_m        = p * mask              (DVE fused multiply, bf16)
  ctx_num    = sum_s p_m[s]*encT[:,s]  (DVE STT-accum over the SAME
                SBUF-resident encT tiles, two s-halves -> 1 HBM pass)
Host post-work: normalize (softmax denominator), scatter packed weights back
to [B, S], reassemble context from the [128, EC] partials.
"""

import numpy as np
import ml_dtypes

B, S, E, H = 64, 2048, 1024, 512
N_CORES = 8
BPC = B // N_CORES        # batches per core
EC = E // 128             # e-chunks (8)
HC = H // 128             # h-chunks (4)
SW = 512                  # matmul moving free dim
NIDX = 1280               # padded kept-position count (mean 1024, sigma ~23)

BF16 = ml_dtypes.bfloat16

_cached = {}


def _build_nc(nidx):
    import concourse.bass as bass  # noqa: F401
    import concourse.tile as tile
    from concourse import bacc, mybir

    f32 = mybir.dt.float32
    bf16 = mybir.dt.bfloat16

    s_tiles = []
    off = 0
    while off < nidx:
        w = min(SW, nidx - off)
        s_tiles.append((off, w))
        off += w
    NSPLIT = len(s_tiles)

    nc = bacc.Bacc("TRN2", target_bir_lowering=False, debug=False,
                   num_devices=N_CORES)

    enc_d = nc.dram_tensor("encT", [BPC, E, nidx], bf16, kind="ExternalInput")
    wh_d = nc.dram_tensor("w_h", [E, H], bf16, kind="ExternalInput")
    wd_d = nc.dram_tensor("w_d", [H, H], f32, kind="ExternalInput")
    decT_d = nc.dram_tensor("decT", [H, BPC], f32, kind="ExternalInput")
    v_d = nc.dram_tensor("v_col", [128, HC], bf16, kind="ExternalInput")
    mask_d = nc.dram_tensor("mask_f", [BPC, nidx], f32, kind="ExternalInput")
    octx_d = nc.dram_tensor("out_ctx", [BPC, NSPLIT, 128, EC], f32,
                            kind="ExternalOutput")
    ow_d = nc.dram_tensor("out_w", [BPC, nidx], bf16, kind="ExternalOutput")

    with tile.TileContext(nc) as tc, \
         tc.tile_pool(name="const", bufs=1) as const_pool, \
         tc.tile_pool(name="enc", bufs=2) as enc_pool, \
         tc.tile_pool(name="energy", bufs=3) as energy_pool, \
         tc.tile_pool(name="small", bufs=2) as small_pool, \
         tc.tile_pool(name="wb", bufs=2) as wb_pool, \
         tc.tile_pool(name="psA", bufs=4, space="PSUM") as psA, \
         tc.tile_pool(name="psB", bufs=2, space="PSUM") as psB:

        # ---- batch-0-critical data first so PE can start ASAP ----
        w_sb = const_pool.tile([128, EC, H], bf16, tag="w_sb")
        nc.sync.dma_start(w_sb[:], wh_d.ap().rearrange("(c p) h -> p c h", p=128))
        enc_g0 = enc_pool.tile([128, EC, nidx], bf16, tag="enc", name="enc_0")
        for ec in range(EC):
            nc.sync.dma_start(enc_g0[:, ec, :], enc_d[0, ec * 128:(ec + 1) * 128, :])

        # ---- remaining constants ----
        wd_sb = const_pool.tile([128, HC, H], f32, tag="wd_sb")
        nc.sync.dma_start(wd_sb[:], wd_d.ap().rearrange("(c p) h -> p c h", p=128))
        decT_sb = const_pool.tile([128, HC, BPC], f32, tag="decT_sb")
        nc.sync.dma_start(decT_sb[:], decT_d.ap().rearrange("(c p) b -> p c b", p=128))
        v_sb = const_pool.tile([128, HC], bf16, tag="v_sb")
        nc.sync.dma_start(v_sb[:], v_d[:, :])
        pdT_sb = const_pool.tile([128, HC, BPC], f32, tag="pdT_sb")

        def emit_pd():
            # proj_dec = (dec @ W_d)^T -> [128, hc, b] (bias for tanh).
            # Emitted after batch 0's first proj tiles so the PE doesn't
            # stall on the wd/decT DMAs at kernel start.
            for hc in range(HC):
                pd_ps = psB.tile([128, BPC], f32, tag="pd", bufs=1,
                                 name=f"pd_{hc}")
                for dc in range(HC):
                    nc.tensor.matmul(
                        pd_ps[:],
                        lhsT=wd_sb[:, dc, hc * 128:(hc + 1) * 128],
                        rhs=decT_sb[:, dc, :],
                        start=(dc == 0), stop=(dc == HC - 1),
                    )
                nc.scalar.copy(pdT_sb[:, hc, :], pd_ps[:])

        # ---- per batch ----
        for b in range(BPC):
            if b == 0:
                enc_g = enc_g0
            else:
                enc_g = enc_pool.tile([128, EC, nidx], bf16, tag="enc",
                                      name=f"enc_{b}")
                for ec in range(EC):
                    nc.sync.dma_start(enc_g[:, ec, :],
                                      enc_d[b, ec * 128:(ec + 1) * 128, :])

            mask_t = small_pool.tile([1, nidx], f32, tag="mask")
            nc.sync.dma_start(mask_t[:], mask_d[b:b + 1, :])

            p_mb = small_pool.tile([1, nidx], bf16, tag="p_mb")
            ctx = small_pool.tile([128, NSPLIT, EC], f32, tag="ctx")
            p_exp = small_pool.tile([1, nidx], f32, tag="p_exp")

            for st, (s0, sw) in enumerate(s_tiles):
                # proj + tanh -> energyT [128, hc, sw] bf16
                energy = energy_pool.tile([128, HC, sw], bf16, tag="energy",
                                          name=f"energy_{b}_{st}",
                                          padded_shape=[128, HC, SW])
                for hc in range(HC):
                    pj = psA.tile([128, sw], f32, tag="pj",
                                  padded_shape=[128, SW])
                    for ec in range(EC):
                        nc.tensor.matmul(
                            pj[:],
                            lhsT=w_sb[:, ec, hc * 128:(hc + 1) * 128],
                            rhs=enc_g[:, ec, s0:s0 + sw],
                            start=(ec == 0), stop=(ec == EC - 1),
                        )
                    if b == 0 and st == 0 and hc == 0:
                        emit_pd()
                    nc.scalar.activation(
                        energy[:, hc, :], pj[:],
                        mybir.ActivationFunctionType.Tanh,
                        bias=pdT_sb[:, hc, b:b + 1], scale=1.0,
                    )
                # scores -> exp
                sc = psB.tile([1, sw], f32, tag="sc", padded_shape=[1, SW])
                for hc in range(HC):
                    nc.tensor.matmul(
                        sc[:],
                        lhsT=v_sb[:, hc:hc + 1],
                        rhs=energy[:, hc, :],
                        start=(hc == 0), stop=(hc == HC - 1),
                    )
                nc.scalar.activation(
                    p_exp[:, s0:s0 + sw], sc[:],
                    mybir.ActivationFunctionType.Exp,
                )

                # masked unnormalized weights (bf16) for this s-tile
                nc.vector.scalar_tensor_tensor(
                    out=p_mb[:, s0:s0 + sw], in0=p_exp[:, s0:s0 + sw],
                    scalar=1.0, in1=mask_t[:, s0:s0 + sw],
                    op0=mybir.AluOpType.bypass, op1=mybir.AluOpType.mult,
                )
                wb = wb_pool.tile([128, sw], bf16, tag="wb",
                                  name=f"wb_{b}_{st}",
                                  padded_shape=[128, SW], bufs=3)
                nc.gpsimd.partition_broadcast(wb[:], p_mb[:, s0:s0 + sw])

                # context partials for this s-tile
                scr = wb_pool.tile([128, sw], bf16, tag="scr",
                                   name=f"scr_{b}_{st}",
                                   padded_shape=[128, SW], bufs=2)
                for ec in range(EC):
                    nc.vector.scalar_tensor_tensor(
                        out=scr[:],
                        in0=enc_g[:, ec, s0:s0 + sw],
                        scalar=1.0,
                        in1=wb[:],
                        op0=mybir.AluOpType.bypass, op1=mybir.AluOpType.mult,
                        accum_out=ctx[:, st, ec:ec + 1],
                    )
                nc.sync.dma_start(octx_d[b, st], ctx[:, st, :])

            nc.sync.dma_start(ow_d[b:b + 1, :], p_mb[:])

    nc.finalize()
    return nc


def _get_nc(nidx):
    key = f"nc_{nidx}"
    if key not in _cached:
        _cached[key] = _build_nc(nidx)
    return _cached[key]


def _host_prep(decoder_hidden, encoder_outputs, mask, W_h, W_d, v, nidx):
    decoder_hidden = np.asarray(decoder_hidden, dtype=np.float32)
    encoder_outputs = np.asarray(encoder_outputs)
    W_h = np.asarray(W_h, dtype=np.float32)
    W_d = np.asarray(W_d, dtype=np.float32)
    v = np.asarray(v, dtype=np.float32)

    wh_bf = np.ascontiguousarray(W_h.astype(BF16))
    v_col = np.ascontiguousarray(v.reshape(HC, 128).T.astype(BF16))

    from concurrent.futures import ThreadPoolExecutor

    def make_core(c):
        sl = slice(c * BPC, (c + 1) * BPC)
        encc = encoder_outputs[sl]
        encT = np.zeros((BPC, E, nidx), dtype=BF16)
        mask_f = np.zeros((BPC, nidx), dtype=np.float32)
        kept_c = []
        for b in range(BPC):
            kept = np.flatnonzero(mask[c * BPC + b])
            n = len(kept)
            kept_c.append(kept)
            encT[b, :, :n] = encc[b, kept].astype(BF16).T
            mask_f[b, :n] = 1.0
        return {
            "encT": encT,
            "w_h": wh_bf,
            "w_d": W_d,
            "decT": np.ascontiguousarray(decoder_hidden[sl].T),
            "v_col": v_col,
            "mask_f": mask_f,
        }, kept_c

    with ThreadPoolExecutor(max_workers=8) as ex:
        out = list(ex.map(make_core, range(N_CORES)))
    in_maps = [o[0] for o in out]
    kept_all = [k for o in out for k in o[1]]
    return in_maps, kept_all


def kernel(decoder_hidden, encoder_outputs, mask, W_h, W_d, v, _trace=False):
    from concourse.bass_utils import run_bass_kernel_spmd

    mask = np.asarray(mask)
    max_kept = int((mask != 0).sum(axis=1).max())
    nidx = NIDX if max_kept <= NIDX else S
    nc = _get_nc(nidx)
    in_maps, kept_all = _host_prep(
        decoder_hidden, encoder_outputs, mask, W_h, W_d, v, nidx)
    res = run_bass_kernel_spmd(nc, in_maps, core_ids=list(range(N_CORES)),
                               trace=_trace)
    # ctx partials: [BPC, 2, 128, EC] -> sum halves -> [e = ec*128 + p]
    ctx_raw = np.concatenate(
        [res.results[i]["out_ctx"].reshape(BPC, -1, 128, EC)
         for i in range(N_CORES)], axis=0)
    ctx_num = ctx_raw.sum(axis=1)                      # [B, 128, EC]
    ctx_num = ctx_num.transpose(0, 2, 1).reshape(B, E)  # e = ec*128 + p
    w_packed = np.concatenate(
        [res.results[i]["out_w"].reshape(BPC, nidx).astype(np.float32)
         for i in range(N_CORES)], axis=0)
    lsum = w_packed.sum(axis=1, keepdims=True)
    context = (ctx_num / lsum).astype(np.float32)
    w_norm = w_packed / lsum
    attn = np.zeros((B, S), dtype=np.float32)
    for b in range(B):
        kept = kept_all[b]
        attn[b, kept] = w_norm[b, :len(kept)]
    if _trace:
        _cached["last_result"] = res
    return context, attn
